# revision 15
# baseline (speedup 1.0000x reference)
"""Multi-head attention (B=4, S=2048, D=1024, H=16) on 8 TRN2 NeuronCores.

Sharding: core c handles batch b = c//2 and head-group hg = c%2 (8 heads).
Tensor-parallel within the core pair of a batch: w_q/w_k/w_v column-split,
w_o row-split; host sums the two partial out-projections per batch.

v3 design (vs baseline):
- Bias elimination: softmax is invariant to k-constant shifts, so the K bias
  is dropped entirely and the V bias is folded into b_o on the host
  (b_o + w_o @ b_v); only the Q bias survives, added on the Vector engine
  (tensor_scalar per-partition add) so ScalarE does nothing but exp.
- AV uses column-tiled concurrent matmuls: head A -> PE cols 0-63,
  head B -> cols 64-127, each K=128, M=64, own moving stream -> the pair's
  attn@V costs one 512-cycle pass per k-tile instead of two.
- Softmax denominators: DVE accumulates et tiles into a 4-way tree
  (bf16), one ones-matmul per (pair, chunk) reduces over partitions
  (two col-tiled M=1 matmuls at PE cols 0 / 32), reciprocal + broadcast-DMA
  + one PSUM*SBUF multiply normalizes straight out of the accumulator.
- Loop order ch-outer/pair-inner; out-projection for chunk ch is woven into
  the attention of ch+1; K o-tiles 1-3, all of V, and later Q chunks are
  woven under the ACT-saturated attention stream so the ScalarE exp pipe
  (the 255us roofline here) starts ~15us into the kernel and never waits
  on a projection phase.
"""

import numpy as np
import ml_dtypes
from contextlib import ExitStack

import concourse.bass as bass
import concourse.tile as tile
from concourse import bacc, mybir
from concourse.bass_utils import run_bass_kernel_spmd

BF16 = ml_dtypes.bfloat16
F32 = np.float32

D = 1024
N_HEAD = 16
DH = 64
HPC = 8          # heads per core
HW = HPC * DH    # head-group width = 512
P = 128

TRACE = False    # set by test.py for profiling runs

_PROG = {}


def _bcast_dma(nc, dst, src_row, engine=None):
    """Broadcast a [1, W] SBUF row to [N, W] via a 0-stride free-dim DMA."""
    n = dst.shape[0]
    src_b = bass.AP(tensor=src_row.tensor, offset=src_row.offset,
                    ap=[list(src_row.ap[0]), [0, n], list(src_row.ap[1])])
    (engine or nc.sync).dma_start(dst, src_b)


def _build_program(S):
    dt = mybir.dt
    bf = dt.bfloat16
    f32 = dt.float32

    CH = min(512, S)         # q-chunk width
    NCH = S // CH            # q-chunks (4)
    NT = S // P              # k-tiles (16)
    NI = D // P              # contraction tiles over model dim (8)
    NP = HPC // 2            # head pairs (4)
    NO = D // P              # out-proj o-tiles (8)
    NOQ = HW // P            # q/k-proj o-tiles (4)

    nc = bacc.Bacc("TRN2", target_bir_lowering=False, debug=False)

    xq = nc.dram_tensor("xq", [D, S], bf, kind="ExternalInput")
    xk = nc.dram_tensor("xk", [D, S], bf, kind="ExternalInput")
    xv = nc.dram_tensor("xv", [D, S], bf, kind="ExternalInput")
    wq = nc.dram_tensor("wq", [D, HW], bf, kind="ExternalInput")
    wk = nc.dram_tensor("wk", [D, HW], bf, kind="ExternalInput")
    wv = nc.dram_tensor("wv", [D, HW], bf, kind="ExternalInput")
    bq = nc.dram_tensor("bq", [P, NOQ], f32, kind="ExternalInput")
    wo = nc.dram_tensor("wo", [HW, D], bf, kind="ExternalInput")
    yT = nc.dram_tensor("yT", [D, S], f32, kind="ExternalOutput")

    AF = mybir.ActivationFunctionType

    with tile.TileContext(nc) as tc:
        with ExitStack() as ctx:
            consts = ctx.enter_context(tc.tile_pool(name="consts", bufs=1))
            wpool = ctx.enter_context(tc.tile_pool(name="wpool", bufs=1))
            xpool = ctx.enter_context(tc.tile_pool(name="xpool", bufs=8))
            slabs = ctx.enter_context(tc.tile_pool(name="slabs", bufs=1))
            epool = ctx.enter_context(tc.tile_pool(name="epool", bufs=6))
            espool = ctx.enter_context(tc.tile_pool(name="espool", bufs=4))
            npool = ctx.enter_context(tc.tile_pool(name="npool", bufs=1))
            spool = ctx.enter_context(tc.tile_pool(name="spool", bufs=2))
            pssc = ctx.enter_context(
                tc.tile_pool(name="pssc", bufs=2, space="PSUM"))
            psmix = ctx.enter_context(
                tc.tile_pool(name="psmix", bufs=1, space="PSUM"))

            # ---- constants ----
            ones1 = consts.tile([P, 1], bf)
            nc.vector.memset(ones1[:], 1.0)
            bq_sb = consts.tile([P, NOQ], f32)
            nc.sync.dma_start(bq_sb[:], bq.ap())

            # ---- weights ----
            wq_sb = wpool.tile([P, NI, HW], bf)
            wk_sb = wpool.tile([P, NI, HW], bf)
            wv_sb = wpool.tile([P, NI, HW], bf)
            wo_sb = wpool.tile([P, NOQ, D], bf)

            # ---- persistent activation slabs ----
            k_slab = slabs.tile([P, NP, S], bf)
            v_sb = slabs.tile([P, NT, HW], bf)
            # double-buffered over chunks (ring on ch % 2)
            q_slab = slabs.tile([P, NOQ, 2, CH], bf)
            attn_sb = [slabs.tile([P, 2, CH], bf, name=f"attn{pp}")
                       for pp in range(NP)]

            # ---------- DMA loads (separate tags: all three persist) ----------
            xk_t = [xpool.tile([P, S], bf, tag="xk", name=f"xkt{i}")
                    for i in range(NI)]
            for i in range(NI):
                nc.sync.dma_start(wk_sb[:, i, :], wk.ap()[i * P:(i + 1) * P, :])
                nc.sync.dma_start(xk_t[i][:], xk.ap()[i * P:(i + 1) * P, :])
            xq_t = [xpool.tile([P, S], bf, tag="xq", name=f"xqt{i}")
                    for i in range(NI)]
            for i in range(NI):
                nc.sync.dma_start(wq_sb[:, i, :], wq.ap()[i * P:(i + 1) * P, :])
                nc.sync.dma_start(xq_t[i][:], xq.ap()[i * P:(i + 1) * P, :])
            xv_t = [xpool.tile([P, S], bf, tag="xv", name=f"xvt{i}")
                    for i in range(NI)]
            for i in range(NI):
                nc.sync.dma_start(wv_sb[:, i, :], wv.ap()[i * P:(i + 1) * P, :])
                nc.sync.dma_start(xv_t[i][:], xv.ap()[i * P:(i + 1) * P, :])
            for c in range(NOQ):
                nc.sync.dma_start(wo_sb[:, c, :], wo.ap()[c * P:(c + 1) * P, :])

            # ---------- projection helpers (one [128, CH] tile each) ----------
            def kproj(o, chk):
                csl = slice(chk * CH, (chk + 1) * CH)
                ps = psmix.tile([P, CH], f32, tag="mix", name="kp")
                for i in range(NI):
                    nc.tensor.matmul(ps[:], lhsT=wk_sb[:, i, o * P:(o + 1) * P],
                                     rhs=xk_t[i][:, csl],
                                     start=(i == 0), stop=(i == NI - 1))
                nc.vector.tensor_copy(k_slab[:, o, csl], ps[:])

            def qproj(o, chk):
                csl = slice(chk * CH, (chk + 1) * CH)
                ps = psmix.tile([P, CH], f32, tag="mix", name="qp")
                for i in range(NI):
                    nc.tensor.matmul(ps[:], lhsT=wq_sb[:, i, o * P:(o + 1) * P],
                                     rhs=xq_t[i][:, csl],
                                     start=(i == 0), stop=(i == NI - 1))
                nc.vector.tensor_scalar(q_slab[:, o, chk % 2, :], ps[:],
                                        bq_sb[:, o:o + 1], None,
                                        mybir.AluOpType.add)

            def vproj(t):
                tsl = slice(t * P, (t + 1) * P)
                ps = psmix.tile([P, HW], f32, tag="mix", name="vp")
                for i in range(NI):
                    nc.tensor.matmul(ps[:], lhsT=xv_t[i][:, tsl],
                                     rhs=wv_sb[:, i, :],
                                     start=(i == 0), stop=(i == NI - 1))
                nc.vector.tensor_copy(v_sb[:, t, :], ps[:])

            def oproj(o, chk):
                csl = slice(chk * CH, (chk + 1) * CH)
                ps = psmix.tile([P, CH], f32, tag="mix", name="op")
                for c in range(NOQ):
                    nc.tensor.matmul(ps[:], lhsT=wo_sb[:, c, o * P:(o + 1) * P],
                                     rhs=attn_sb[c][:, chk % 2, :],
                                     start=(c == 0), stop=(c == NOQ - 1))
                st = spool.tile([P, CH], f32, tag="stage")
                nc.vector.tensor_copy(st[:], ps[:])
                nc.sync.dma_start(yT.ap()[o * P:(o + 1) * P, csl], st[:])

            # ---------- prologue ----------
            for chk in range(NCH):
                kproj(0, chk)
            for o in range(NOQ):
                qproj(o, 0)

            # weave map kt -> [closures] emitted into the attention kt loops.
            # ch0/p0 carries ALL of V (AV(p0,kt) needs v tile kt in-order)
            # plus K o1; later ch0 pairs carry K o2/o3 + Q; chunks >=1 carry
            # out-proj(ch-1) + Q(ch+1).
            def weave_for(ch, p):
                wv_at = {}

                def put(kt, fn):
                    wv_at.setdefault(kt, []).append(fn)

                if ch == 0:
                    if p == 0:
                        # all of V: AV(p0, kt) consumes v tile kt in-order
                        for t in range(NT):
                            put(t, lambda tt=t: vproj(tt))
                        for chk in range(NCH):
                            put(2 * chk + 1, lambda c=chk: kproj(1, c))
                    elif p < NP - 1:
                        # K o-tile p+1 must land before pair p+1 starts
                        for chk in range(NCH):
                            put(2 * chk, lambda o=p + 1, c=chk: kproj(o, c))
                else:
                    # out-projection of the previous chunk: 2 o-tiles per pair
                    put(0, lambda o=2 * p, c=ch - 1: oproj(o, c))
                    put(4, lambda o=2 * p + 1, c=ch - 1: oproj(o, c))
                if ch < NCH - 1:
                    # Q o-tile p of the next chunk, one chunk ahead
                    put(12, lambda o=p, c=ch + 1: qproj(o, c))
                return wv_at

            # ---------- attention ----------
            for ch in range(NCH):
                csl2 = slice(ch * CH, (ch + 1) * CH)
                for p in range(NP):
                    wv_at = weave_for(ch, p)

                    acc = pssc.tile([P, CH], f32, tag="acc", bufs=3,
                                    name="acc")
                    esacc = [espool.tile([P, 2 * CH], bf, tag="es",
                                         name=f"es{j}") for j in range(4)]
                    pend = []

                    def issue_av(et, kt):
                        vb = v_sb[:, kt, p * P:p * P + 64]
                        nc.tensor.matmul(
                            acc[0:64, :], lhsT=vb, rhs=et[:, 0:CH],
                            start=(kt == 0), stop=(kt == NT - 1),
                            tile_position=(0, 0))
                        vb2 = v_sb[:, kt, p * P + 64:(p + 1) * P]
                        nc.tensor.matmul(
                            acc[64:128, :], lhsT=vb2, rhs=et[:, CH:2 * CH],
                            start=(kt == 0), stop=(kt == NT - 1),
                            tile_position=(0, 64))

                    for kt in range(NT):
                        ksl = slice(kt * P, (kt + 1) * P)
                        ps = pssc.tile([P, 2 * CH], f32, tag="sc", name="sc")
                        nc.tensor.matmul(
                            ps[:, 0:CH],
                            lhsT=k_slab[0:64, p, ksl],
                            rhs=q_slab[0:64, p, ch % 2, :],
                            start=True, stop=True, tile_position=(0, 0))
                        nc.tensor.matmul(
                            ps[:, CH:2 * CH],
                            lhsT=k_slab[64:128, p, ksl],
                            rhs=q_slab[64:128, p, ch % 2, :],
                            start=True, stop=True, tile_position=(64, 0))
                        et = epool.tile([P, 2 * CH], bf, tag="exp", name="et")
                        nc.scalar.activation(et[:], ps[:], AF.Exp, scale=0.125)
                        # DVE denominator accumulation (4-way tree)
                        j = kt % 4
                        if kt < 4:
                            nc.vector.tensor_copy(esacc[j][:], et[:])
                        else:
                            nc.vector.tensor_add(esacc[j][:], esacc[j][:],
                                                 et[:])
                        pend.append((et, kt))
                        if len(pend) == 3:
                            e0, k0 = pend.pop(0)
                            issue_av(e0, k0)
                        for item in wv_at.get(kt, ()):
                            item()
                    for e0, k0 in pend:
                        issue_av(e0, k0)

                    # finish denominator tree + partition reduction
                    nc.vector.tensor_add(esacc[0][:], esacc[0][:], esacc[1][:])
                    nc.vector.tensor_add(esacc[2][:], esacc[2][:], esacc[3][:])
                    nc.vector.tensor_add(esacc[0][:], esacc[0][:], esacc[2][:])
                    psda = psmix.tile([1, CH], f32, tag="mix", name="psda")
                    nc.tensor.matmul(psda[0:1, :], lhsT=ones1[:],
                                     rhs=esacc[0][:, 0:CH],
                                     start=True, stop=True,
                                     tile_position=(0, 0))
                    psdb = psmix.tile([1, CH], f32, tag="mix", name="psdb")
                    nc.tensor.matmul(psdb[0:1, :], lhsT=ones1[:],
                                     rhs=esacc[0][:, CH:2 * CH],
                                     start=True, stop=True,
                                     tile_position=(0, 0))

                    # normalize: recip of the two denom rows (straight from
                    # PSUM), broadcast, multiply out of the PSUM accumulator
                    rcpa = npool.tile([1, CH], f32, tag="rcpa")
                    nc.vector.reciprocal_approx_fast(rcpa[:], psda[0:1, :])
                    rcpb2 = npool.tile([1, CH], f32, tag="rcpb2")
                    nc.vector.reciprocal_approx_fast(rcpb2[:], psdb[0:1, :])
                    rcpb = npool.tile([P, CH], f32, tag="rcpb")
                    _bcast_dma(nc, rcpb[0:64, :], rcpa[0:1, :],
                               engine=nc.gpsimd)
                    _bcast_dma(nc, rcpb[64:128, :], rcpb2[0:1, :],
                               engine=nc.gpsimd)
                    nc.vector.tensor_mul(attn_sb[p][:, ch % 2, :], acc[:],
                                         rcpb[:])

            # ---------- tail: out-projection of the last chunk ----------
            for o in range(NO):
                oproj(o, NCH - 1)

    nc.compile()
    return nc


def _get_program(S):
    if S not in _PROG:
        _PROG[S] = _build_program(S)
    return _PROG[S]


def enable_trace():
    """Register the NTFF profiling hook (axon images lack antenv.axon_hooks)
    and neuter the cloud artifact upload; then TRACE=True runs return
    exec_time_ns."""
    global TRACE
    import sys
    import types
    import antenv
    if "antenv.axon_hooks" not in sys.modules:
        _m = types.ModuleType("antenv.axon_hooks")
        _m._hook = None
        _m.set_axon_ntff_profile_hook = lambda h: setattr(_m, "_hook", h)
        _m.get_axon_ntff_profile_hook = lambda: _m._hook
        sys.modules["antenv.axon_hooks"] = _m
        antenv.axon_hooks = _m
        from trn_agent_boot.trn_boot import _ntff_profile_via_ctypes
        _m._hook = _ntff_profile_via_ctypes("/opt/axon/libaxon_pjrt.so")
    import concourse.bass_utils as bu
    bu.upload_artifacts = lambda tmpdir: tmpdir
    TRACE = True


def _prep_core_inputs(q, k, v, w_q, b_q, w_k, w_v, b, hg, S):
    hsl = slice(hg * HW, (hg + 1) * HW)
    return {
        "xq": np.ascontiguousarray(q[b].T).astype(BF16),
        "xk": np.ascontiguousarray(k[b].T).astype(BF16),
        "xv": np.ascontiguousarray(v[b].T).astype(BF16),
        "wq": np.ascontiguousarray(w_q[hsl].T).astype(BF16),
        "wk": np.ascontiguousarray(w_k[hsl].T).astype(BF16),
        "wv": np.ascontiguousarray(w_v[hsl].T).astype(BF16),
        "bq": np.ascontiguousarray(b_q[hsl].reshape(HW // P, P).T).astype(F32),
    }


def kernel(q, k, v, w_q, b_q, w_k, b_k, w_v, b_v, w_o, b_o):
    q, k, v = (np.asarray(a, F32) for a in (q, k, v))
    w_q, b_q, w_k, b_k = (np.asarray(a, F32) for a in (w_q, b_q, w_k, b_k))
    w_v, b_v, w_o, b_o = (np.asarray(a, F32) for a in (w_v, b_v, w_o, b_o))
    B, S, _ = q.shape

    nc = _get_program(S)

    # softmax(s + const_over_k) == softmax(s): b_k drops out entirely.
    # b_v contributes attn @ 1 * b_v = b_v (rows sum to 1), folded into b_o.
    b_o_eff = b_o + w_o @ b_v

    n_cores = 2 * B
    in_maps = []
    for c in range(n_cores):
        b, hg = c // 2, c % 2
        m = _prep_core_inputs(q, k, v, w_q, b_q, w_k, w_v, b, hg, S)
        hsl = slice(hg * HW, (hg + 1) * HW)
        m["wo"] = np.ascontiguousarray(w_o[:, hsl].T).astype(BF16)
        in_maps.append(m)

    res = run_bass_kernel_spmd(nc, in_maps, list(range(n_cores)), trace=TRACE)

    out = np.empty((B, S, D), F32)
    for b in range(B):
        yt = res.results[2 * b]["yT"] + res.results[2 * b + 1]["yT"]
        out[b] = yt.T + b_o_eff
    if TRACE:
        kernel.last_exec_time_ns = res.exec_time_ns
    return out
